# revision 20
# baseline (speedup 1.0000x reference)
"""Lennard-Jones pair energies + per-atom segment sum on 8 Trainium2 cores.

Strategy (edge-partitioned per the sharding hint, ELL-style dense layout):

Host (sharding step): atoms are sorted by padded pair count and grouped into
chunks of 1024 (8 cores x 128 partitions); chunk i keeps L_i = max padded
count in the chunk, so there are no pad atoms and slot padding is minimal.
Each core receives a partition-major dense buffer [128, F_total] where
partition p's row is the concatenation over chunks of that atom's L_i-slot
run -- every DMA is fully contiguous per partition.  Pad slots use dist=RC
(shifted LJ energy exactly 0).  Slot 0 of each chunk is a host-computed
fixup distance d_L whose pair energy equals the column's additive constant
-L*e0/2, so the device-side reduce alone yields the final per-atom energy.

Device: one activation-table preload (ln/exp/square share a table set), then
per tile of ~1.1K columns: contiguous DMA, ACT ln, ACT exp (v = sqrt2*d^-6),
GPSIMD stt bp = (v - 2b)*v with b = sqrt(1/2)  (en/2 = bp - e0/2 per pair),
and DVE grouped tensor_reduce over each equal-L chunk run into the per-atom
output column.  One final DMA writes [128, n_chunks] back to HBM.

Host (unshard step): scatters per-atom results back to atom order.
"""

import math

import numpy as np

RC = 3.0
N_CORES = 8
P = 128
CH = N_CORES * P  # atoms per chunk
PAD_MULT = 2  # per-atom slot-count quantum

_E0 = 4.0 * ((1.0 / RC) ** 12 - (1.0 / RC) ** 6)
_B = math.sqrt(0.5)


def _merge_runs(Lc: np.ndarray, max_runs: int = 7, max_cost: int = 60000):
    """Round some chunks' L up to the next-larger run's L to cut the number
    of distinct L values. Lc is non-increasing (sorted desc)."""
    Lc = Lc.copy()
    while True:
        uniq = sorted(set(int(x) for x in Lc), reverse=True)
        if len(uniq) <= max_runs:
            break
        best = None
        for i in range(1, len(uniq)):
            src = uniq[i]
            dst = uniq[i - 1]
            m = int(np.sum(Lc == src))
            cost = m * CH * (dst - src)
            if best is None or cost < best[0]:
                best = (cost, src, dst)
        if best[0] > max_cost:
            break
        Lc[Lc == best[1]] = best[2]
    return Lc


def _build_layout(idx: np.ndarray, n_atoms: int, dist: np.ndarray):
    """Pack pairs into per-core partition-major ELL chunks.

    Returns (packed, atom_of, Lp, n_chunks):
      packed:  [N_CORES, P, F_total] f32 device input
      atom_of: [n_chunks, N_CORES, P] atom id per output cell (-1 = pad)
      Lp:      per-chunk padded width incl. fixup slot
    """
    counts = np.bincount(idx, minlength=n_atoms).astype(np.int64)
    perm = np.argsort(idx, kind="stable")
    starts = np.zeros(n_atoms + 1, np.int64)
    starts[1:] = np.cumsum(counts)
    q = ((counts + PAD_MULT - 1) // PAD_MULT) * PAD_MULT
    order = np.argsort(-q, kind="stable")
    n_chunks = (n_atoms + CH - 1) // CH
    n_pad = n_chunks * CH
    order_pad = np.full(n_pad, -1, np.int64)
    order_pad[:n_atoms] = order
    qs = np.where(order_pad >= 0, q[np.maximum(order_pad, 0)], 0)
    Lc = np.maximum(qs.reshape(n_chunks, CH).max(axis=1), PAD_MULT)
    # merge small equal-L runs upward (fewer distinct L values -> fewer
    # device reduce instructions) while the slot-padding cost stays tiny
    Lc = _merge_runs(Lc)
    Lp = Lc + 1  # fixup slot
    col0 = np.zeros(n_chunks + 1, np.int64)
    col0[1:] = np.cumsum(Lp)
    F_total = int(col0[-1])

    # fixup distance per chunk: bp(d_L) = -Lc*e0/2
    vfix = _B + np.sqrt(0.5 - Lc * _E0 / 2.0)
    dfix = (math.sqrt(2.0) / vfix) ** (1.0 / 6.0)

    dist_sorted = dist[perm].astype(np.float16)
    packed = np.full((N_CORES, P, F_total), np.float16(RC), np.float16)
    offs_max = np.arange(int(Lc.max()))
    for i in range(n_chunks):
        a = order_pad[i * CH : (i + 1) * CH]
        L = int(Lc[i])
        o = int(col0[i])
        cnt = np.where(a >= 0, counts[np.maximum(a, 0)], 0)
        offs = offs_max[:L][None, :]
        valid = offs < cnt[:, None]
        src = starts[np.maximum(a, 0)][:, None] + offs
        block = np.full((CH, L), np.float16(RC), np.float16)
        block[valid] = dist_sorted[src[valid]]
        packed[:, :, o + 1 : o + 1 + L] = block.reshape(N_CORES, P, L)
        packed[:, :, o] = np.float16(dfix[i])
    atom_of = order_pad.reshape(n_chunks, N_CORES, P)
    return packed, atom_of, [int(x) for x in Lp], n_chunks


def _tile_plan(Lp):
    """Group chunks into device tiles with a graded size schedule (small
    first tile for fast pipeline ramp, small last tile for a short tail).

    Returns list of tiles; each tile is (col_start, F, runs) with
    runs = [(tile_col_off, L, m, out_col)] for maximal equal-L chunk runs.
    """
    n = len(Lp)
    total = sum(Lp)
    # target cumulative boundaries as fractions of total width
    fracs = [0.04, 0.16, 0.36, 0.58, 0.80, 1.0]
    bounds = []
    c0 = 0
    width = 0
    col = 0
    fi = 0
    for i in range(n):
        width += Lp[i]
        col += Lp[i]
        if fi < len(fracs) - 1 and col >= fracs[fi] * total:
            bounds.append((c0, i + 1))
            c0 = i + 1
            width = 0
            fi += 1
    if c0 < n:
        bounds.append((c0, n))
    tiles = []
    col = 0
    for c0, c1 in bounds:
        runs = []
        off = 0
        j = c0
        while j < c1:
            k = j
            while k < c1 and Lp[k] == Lp[j]:
                k += 1
            runs.append((off, Lp[j], k - j, j))
            off += Lp[j] * (k - j)
            j = k
        tiles.append((col, off, runs))
        col += off
    return tiles


def _build_bass_program(Lp, F_total, n_chunks):
    import concourse.bass as bass
    import concourse.tile as tile
    from concourse import bacc, mybir

    f32 = mybir.dt.float32
    f16 = mybir.dt.float16
    AF = mybir.ActivationFunctionType
    OP = mybir.AluOpType

    nc = bacc.Bacc(
        "TRN2",
        target_bir_lowering=False,
        debug=False,
        enable_asserts=False,
        num_devices=N_CORES,
    )
    din = nc.dram_tensor("dist_packed", [P, F_total], f16, kind="ExternalInput")
    dout = nc.dram_tensor("en_out", [P, n_chunks], f32, kind="ExternalOutput")

    # activation table set holding ln+exp together (one load for the whole
    # program instead of a 1.3us reload per function switch)
    set_id = 6
    try:
        from concourse.hw_specs import get_activation_tables

        for i, (_, funcs) in enumerate(get_activation_tables("TRN2").items()):
            if AF.Ln in funcs and AF.Exp in funcs:
                set_id = i
                break
    except Exception:
        pass

    tiles = _tile_plan(Lp)
    ln_sqrt2 = 0.5 * math.log(2.0)

    with tile.TileContext(nc) as tc:
        with (
            tc.tile_pool(name="io", bufs=4) as io_pool,
            tc.tile_pool(name="t", bufs=2) as tpool,
            tc.tile_pool(name="u", bufs=2) as upool,
            tc.tile_pool(name="acc", bufs=1) as acc_pool,
        ):
            atl = mybir.InstLoadActFuncSet(
                name=nc.get_next_instruction_name(),
                ins=[],
                outs=[],
                act_func_set_id=set_id,
            )
            nc.scalar.add_instruction(atl)
            out_raw = acc_pool.tile([P, n_chunks], f32, tag="out_raw")
            lbias = acc_pool.tile([P, 1], f32, tag="lbias")
            nc.vector.memset(lbias[:], ln_sqrt2)
            for ti, (col, F, runs) in enumerate(tiles):
                d = io_pool.tile([P, F], f16, tag="d")
                nc.sync.dma_start(d[:], din.ap()[:, col : col + F])
                # t = ln(d) at f32 (exp amplifies ln error 6x)
                t = tpool.tile([P, F], f32, tag="t")
                nc.scalar.activation(t[:], d[:], AF.Ln)
                # v = sqrt2*d^-6 in fp16: tensor_scalar runs 4x and
                # tensor_tensor 2x on 2-byte dtypes; overall l2 ~1.6e-3
                nc.scalar.activation(
                    d[:], t[:], AF.Exp, bias=lbias[:], scale=-6.0
                )
                # bp = (v - 2b)*v ; en/2 = bp - e0/2 (constant folded into
                # the per-chunk fixup slot).  Split as ts (4x) + tt (2x)
                # instead of scalar_tensor_tensor (1x on hardware).
                v = d
                u = upool.tile([P, F], f16, tag="u")
                nc.vector.tensor_scalar(u[:], v[:], 2.0 * _B, None, OP.subtract)
                # split the multiply between DVE (2x) and the otherwise-idle
                # GpSimd engine (~0.42 efficiency) to keep DVE below ACT
                sp = int(F * 0.62) & ~1
                nc.vector.tensor_tensor(v[:, :sp], u[:, :sp], v[:, :sp], OP.mult)
                nc.gpsimd.tensor_tensor(v[:, sp:], u[:, sp:], v[:, sp:], OP.mult)
                c0 = runs[0][3]
                c1 = runs[-1][3] + runs[-1][2]
                for off, L, m, out_col in runs:
                    nc.vector.tensor_reduce(
                        out_raw[:, out_col : out_col + m],
                        v[:, off : off + m * L].rearrange(
                            "p (b l) -> p b l", l=L
                        ),
                        axis=mybir.AxisListType.X,
                        op=OP.add,
                    )
                nc.sync.dma_start(
                    dout.ap()[:, c0:c1], out_raw[:, c0:c1]
                )
    nc.compile()
    return nc


def _prepare(inputs):
    dist = np.ascontiguousarray(np.asarray(inputs["dist"], dtype=np.float32))
    ind_2 = np.asarray(inputs["ind_2"])
    n_atoms = int(np.asarray(inputs["ind_1"]).shape[0])
    idx = ind_2[:, 0].astype(np.int64)

    packed, atom_of, Lp, n_chunks = _build_layout(idx, n_atoms, dist)
    F_total = packed.shape[2]
    in_maps = [
        {"dist_packed": np.ascontiguousarray(packed[c])} for c in range(N_CORES)
    ]
    nc = _build_bass_program(Lp, F_total, n_chunks)
    return nc, in_maps, (atom_of, n_atoms)


def _finish(res, meta):
    atom_of, n_atoms = meta
    out_full = np.zeros(n_atoms, np.float32)
    for c in range(N_CORES):
        dev = res.results[c]["en_out"]  # [P, n_chunks]
        a = atom_of[:, c, :]  # [n_chunks, P]
        valid = a >= 0
        out_full[a[valid]] = dev.T[valid]
    return out_full


def kernel(**inputs) -> np.ndarray:
    nc, in_maps, meta = _prepare(inputs)

    from concourse import bass_utils

    res = bass_utils.run_bass_kernel_spmd(
        nc, in_maps, core_ids=list(range(N_CORES))
    )
    return _finish(res, meta)
